# revision 3
# baseline (speedup 1.0000x reference)
"""BitNet FFN (quantized MLP) on 8 Trainium2 NeuronCores.

Reference computation (per problem nn_BitNetModel_18554258719008):
    h   = quant_act(x) @ tern(W1).T + b1 ; h = gelu(h, exact)
    out = quant_act(h) @ tern(W2).T + b2
  quant_act(v) = round(clip(v*s, -127, 127))/s,  s = 127/max|v| (global)
  tern(w)      = round(clip(w/t, -1, 1))*t,      t = mean|w|    (global)

Key fact: after quantization the matmul operands are exact small integers
(activations in [-127,127], weights in {-1,0,1}) -> bf16 matmul is EXACT.

Sharding (8 cores, SPMD):
  - tokens sharded 8-ways: core r owns tokens [r*1024, (r+1)*1024)
  - W1 rows (d_ff) and W2 rows (d_model) sharded 8-ways for the quantize
    work; quantized (bf16) transposed shards are AllGathered so every core
    has the full ternary weights.
  - global scales: one AllReduce(max) for amax(x), one AllReduce(add) for
    (sum|W1|, sum|W2|), one AllReduce(max) for amax(h).
  - all operands are fed pre-transposed by the host (pure layout prep), so
    the contraction dim lands on SBUF partitions with no on-device
    transposes. Each core computes h.T (its token shard) then out.T; the
    host reassembles out from the out.T shards.
"""

import os
import numpy as np

import concourse.bass as bass
import concourse.bacc as bacc_mod
import concourse.mybir as mybir
from concourse.tile import TileContext
from concourse.bass_utils import run_bass_kernel_spmd

P = 128
NCORES = 8
RND = 12582912.0  # 1.5 * 2**23: (v + RND) - RND == round-half-even(v) for |v|<2^22

f32 = mybir.dt.float32
bf16 = mybir.dt.bfloat16
A = mybir.AluOpType
AF = mybir.ActivationFunctionType
AX = mybir.AxisListType


def build(T=8192, D=2048, F=8192, gelu=True):
    """Build the SPMD kernel for one core (same program on all 8)."""
    TOK = T // NCORES    # tokens per core
    FSH = F // NCORES    # d_ff rows per core (W1 shard)
    DSH = D // NCORES    # d_model rows per core (W2 shard)
    KT1 = D // P         # contraction tiles, layer 1
    KT2 = F // P         # contraction tiles, layer 2
    MPS1 = FSH // P      # m-tiles per w1 slab
    MPS2 = DSH // P      # m-tiles per w2 slab
    NF = min(512, TOK)   # matmul moving free width
    NCH = TOK // NF      # chunks of the token axis per matmul row
    KG2 = min(16, KT2)   # k-tiles per w2 slab load group
    G2 = KT2 // KG2
    inv_nw = 1.0 / (D * F)            # 1/numel(W)
    inv_nw127 = 1.0 / (D * F) / 127.0

    nc = bacc_mod.Bacc(num_devices=NCORES)

    # ---- I/O (host supplies transposed shards; see kernel() below)
    xT = nc.dram_tensor("xT", [D, TOK], f32, kind="ExternalInput")
    w1T = nc.dram_tensor("w1T", [D, FSH], f32, kind="ExternalInput")
    w2T = nc.dram_tensor("w2T", [F, DSH], f32, kind="ExternalInput")
    b1t = nc.dram_tensor("b1t", [P, F // P], f32, kind="ExternalInput")
    b2t = nc.dram_tensor("b2t", [P, D // P], f32, kind="ExternalInput")
    outT = nc.dram_tensor("outT", [D, TOK], f32, kind="ExternalOutput")

    # ---- internal DRAM
    h_dram = nc.dram_tensor("h_dram", [F, TOK], f32)
    cc_max_in = nc.dram_tensor("cc_max_in", [1, 8], f32)
    cc_max_out = nc.dram_tensor("cc_max_out", [1, 8], f32, addr_space="Shared")
    cc_sum_in = nc.dram_tensor("cc_sum_in", [1, 8], f32)
    cc_sum_out = nc.dram_tensor("cc_sum_out", [1, 8], f32, addr_space="Shared")
    cc_hmax_in = nc.dram_tensor("cc_hmax_in", [1, 8], f32)
    cc_hmax_out = nc.dram_tensor("cc_hmax_out", [1, 8], f32, addr_space="Shared")
    cc_w1_in = nc.dram_tensor("cc_w1_in", [D, FSH], bf16)
    cc_w1_out = nc.dram_tensor("cc_w1_out", [NCORES * D, FSH], bf16, addr_space="Shared")
    cc_w2_in = nc.dram_tensor("cc_w2_in", [F, DSH], bf16)
    cc_w2_out = nc.dram_tensor("cc_w2_out", [NCORES * F, DSH], bf16, addr_space="Shared")
    scal1_dram = nc.dram_tensor("scal1_dram", [1, 8], f32)
    scal2_dram = nc.dram_tensor("scal2_dram", [1, 8], f32)

    rg = [list(range(NCORES))]

    with TileContext(nc) as tc:
        with (
            tc.tile_pool(name="const", bufs=1) as cpool,
            tc.tile_pool(name="small", bufs=1) as small,
        ):
            # biases (pre-arranged [P, n] by host)
            b1_sb = cpool.tile([P, F // P], f32)
            nc.sync.dma_start(b1_sb[:], b1t[:, :])
            b2_sb = cpool.tile([P, D // P], f32)
            nc.sync.dma_start(b2_sb[:], b2t[:, :])

            # =========================== phase A: local reduces + scale ARs
            with tc.tile_pool(name="ldA", bufs=3) as ldA:
                xmax_cols = small.tile([P, KT1], f32)
                w1sum_cols = small.tile([P, KT1], f32)
                w2sum_cols = small.tile([P, KT2], f32)
                for i in range(KT1):
                    t = ldA.tile([P, TOK], f32, tag="ldx", name=f"ax{i}")
                    nc.sync.dma_start(t[:], xT[i * P:(i + 1) * P, :])
                    nc.vector.tensor_reduce(
                        xmax_cols[:, i:i + 1], t[:], axis=AX.X, op=A.max,
                        apply_absolute_value=True)
                for i in range(KT1):
                    t = ldA.tile([P, FSH], f32, tag="ldw1", name=f"aw1{i}")
                    nc.sync.dma_start(t[:], w1T[i * P:(i + 1) * P, :])
                    nc.vector.tensor_reduce(
                        w1sum_cols[:, i:i + 1], t[:], axis=AX.X, op=A.add,
                        apply_absolute_value=True)
                for i in range(KT2):
                    t = ldA.tile([P, DSH], f32, tag="ldw2", name=f"aw2{i}")
                    nc.sync.dma_start(t[:], w2T[i * P:(i + 1) * P, :])
                    nc.vector.tensor_reduce(
                        w2sum_cols[:, i:i + 1], t[:], axis=AX.X, op=A.add,
                        apply_absolute_value=True)

                xmax_p = small.tile([P, 1], f32)
                nc.vector.tensor_reduce(xmax_p[:], xmax_cols[:], axis=AX.X, op=A.max)
                w1sum_p = small.tile([P, 1], f32)
                nc.vector.tensor_reduce(w1sum_p[:], w1sum_cols[:], axis=AX.X, op=A.add)
                w2sum_p = small.tile([P, 1], f32)
                nc.vector.tensor_reduce(w2sum_p[:], w2sum_cols[:], axis=AX.X, op=A.add)

                # cross-partition reduce on gpsimd, stage for collectives
                stage_max = small.tile([1, 8], f32)
                nc.gpsimd.memset(stage_max[:], 0.0)
                nc.gpsimd.tensor_reduce(stage_max[:, 0:1], xmax_p[:], axis=AX.C, op=A.max)
                nc.sync.dma_start(cc_max_in[:, :], stage_max[:])
                stage_sum = small.tile([1, 8], f32)
                nc.gpsimd.memset(stage_sum[:], 0.0)
                nc.gpsimd.tensor_reduce(stage_sum[:, 0:1], w1sum_p[:], axis=AX.C, op=A.add)
                nc.gpsimd.tensor_reduce(stage_sum[:, 1:2], w2sum_p[:], axis=AX.C, op=A.add)
                nc.sync.dma_start(cc_sum_in[:, :], stage_sum[:])

                nc.gpsimd.collective_compute(
                    "AllReduce", A.max, replica_groups=rg,
                    ins=[cc_max_in[:, :]], outs=[cc_max_out[:, :]])
                nc.gpsimd.collective_compute(
                    "AllReduce", A.add, replica_groups=rg,
                    ins=[cc_sum_in[:, :]], outs=[cc_sum_out[:, :]])

                gmax = small.tile([1, 8], f32)
                nc.sync.dma_start(gmax[:], cc_max_out[:, :])
                gsum = small.tile([1, 8], f32)
                nc.sync.dma_start(gsum[:], cc_sum_out[:, :])

                # scalars: sx=127/amax, r1=1/t1, r2=1/t2, alpha1=t1/sx
                scal = small.tile([1, 8], f32)
                nc.vector.reciprocal(scal[:, 0:1], gmax[:, 0:1])
                nc.vector.tensor_scalar_mul(scal[:, 0:1], scal[:, 0:1], 127.0)
                nc.vector.reciprocal(scal[:, 1:2], gsum[:, 0:1])
                nc.vector.tensor_scalar_mul(scal[:, 1:2], scal[:, 1:2], float(D * F))
                nc.vector.reciprocal(scal[:, 2:3], gsum[:, 1:2])
                nc.vector.tensor_scalar_mul(scal[:, 2:3], scal[:, 2:3], float(D * F))
                nc.vector.tensor_tensor(scal[:, 3:4], gsum[:, 0:1], gmax[:, 0:1], A.mult)
                nc.vector.tensor_scalar_mul(scal[:, 3:4], scal[:, 3:4], inv_nw127)
                nc.sync.dma_start(scal1_dram[:, 0:4], scal[:, 0:4])
                bc1 = cpool.tile([P, 4], f32)
                nc.sync.dma_start(bc1[:], scal1_dram[:, 0:4].to_broadcast((P, 4)))
                sx_b, r1_b, r2_b, a1_b = (bc1[:, i:i + 1] for i in range(4))

            # =========================== phase B: quantize shards, AGs, layer 1
            with (
                tc.tile_pool(name="ldB", bufs=3) as ldB,
                tc.tile_pool(name="qt", bufs=3) as qt,
                tc.tile_pool(name="qo", bufs=3) as qo,
                tc.tile_pool(name="xq", bufs=1) as xqp,
                tc.tile_pool(name="w1s", bufs=2) as w1s,
                tc.tile_pool(name="hst", bufs=3) as hst,
                tc.tile_pool(name="ps1", bufs=2, space="PSUM") as ps1,
            ):
                # --- ternary-quantize W1 shard -> cc_w1_in (bf16)
                for i in range(KT1):
                    t = ldB.tile([P, FSH], f32, tag="lw1", name=f"bw1{i}")
                    nc.sync.dma_start(t[:], w1T[i * P:(i + 1) * P, :])
                    q1 = qt.tile([P, FSH], f32, tag="q1", name=f"bq1{i}")
                    nc.scalar.activation(q1[:], t[:], AF.Copy, scale=r1_b)
                    nc.vector.tensor_scalar(q1[:], q1[:], 1.0, -1.0, A.min, A.max)
                    qb = qo.tile([P, FSH], bf16, tag="qb1", name=f"bqb1{i}")
                    nc.vector.tensor_scalar(qb[:], q1[:], RND, RND, A.add, A.subtract)
                    nc.sync.dma_start(cc_w1_in[i * P:(i + 1) * P, :], qb[:])
                nc.gpsimd.collective_compute(
                    "AllGather", A.bypass, replica_groups=rg,
                    ins=[cc_w1_in[:, :]], outs=[cc_w1_out[:, :]])

                # --- quantize x shard -> resident xq (bf16 ints)
                xq = xqp.tile([P, KT1, TOK], bf16)
                for i in range(KT1):
                    t = ldB.tile([P, TOK], f32, tag="lx", name=f"bx{i}")
                    nc.sync.dma_start(t[:], xT[i * P:(i + 1) * P, :])
                    q1 = qt.tile([P, TOK], f32, tag="qx", name=f"bqx{i}")
                    nc.scalar.activation(q1[:], t[:], AF.Copy, scale=sx_b)
                    nc.vector.tensor_scalar(
                        xq[:, i, :], q1[:], RND, RND, A.add, A.subtract)

                # --- ternary-quantize W2 shard -> cc_w2_in, AG (overlaps layer 1)
                for i in range(KT2):
                    t = ldB.tile([P, DSH], f32, tag="lw2", name=f"bw2{i}")
                    nc.sync.dma_start(t[:], w2T[i * P:(i + 1) * P, :])
                    q1 = qt.tile([P, DSH], f32, tag="q2", name=f"bq2{i}")
                    nc.scalar.activation(q1[:], t[:], AF.Copy, scale=r2_b)
                    nc.vector.tensor_scalar(q1[:], q1[:], 1.0, -1.0, A.min, A.max)
                    qb = qo.tile([P, DSH], bf16, tag="qb2", name=f"bqb2{i}")
                    nc.vector.tensor_scalar(qb[:], q1[:], RND, RND, A.add, A.subtract)
                    nc.sync.dma_start(cc_w2_in[i * P:(i + 1) * P, :], qb[:])
                nc.gpsimd.collective_compute(
                    "AllGather", A.bypass, replica_groups=rg,
                    ins=[cc_w2_in[:, :]], outs=[cc_w2_out[:, :]])

                # --- layer 1: h.T[m-tile] = W1q.T @ xq ; gelu; track amax; spill
                hmax_cols = small.tile([P, F // P], f32)
                for r in range(NCORES):
                    slab = w1s.tile([P, KT1, FSH], bf16, tag="w1slab", name=f"w1s{r}")
                    nc.sync.dma_start(
                        slab[:],
                        cc_w1_out[r * D:(r + 1) * D, :].rearrange(
                            "(k p) f -> p k f", p=P))
                    for j in range(MPS1):
                        m = r * MPS1 + j
                        ps = [
                            ps1.tile([P, NF], f32, tag=f"ps{c}", name=f"ps_{m}_{c}")
                            for c in range(NCH)
                        ]
                        for k in range(KT1):
                            lhsT = slab[:, k, j * P:(j + 1) * P]
                            for c in range(NCH):
                                nc.tensor.matmul(
                                    ps[c][:], lhsT, xq[:, k, c * NF:(c + 1) * NF],
                                    start=(k == 0), stop=(k == KT1 - 1))
                        h_sb = hst.tile([P, TOK], f32, tag="hsb", name=f"h{m}")
                        act_fn = AF.Gelu if gelu else AF.Identity
                        for c in range(NCH):
                            nc.scalar.activation(
                                h_sb[:, c * NF:(c + 1) * NF], ps[c][:], act_fn,
                                bias=b1_sb[:, m:m + 1], scale=a1_b)
                        nc.vector.tensor_reduce(
                            hmax_cols[:, m:m + 1], h_sb[:], axis=AX.X, op=A.max,
                            apply_absolute_value=True)
                        nc.sync.dma_start(h_dram[m * P:(m + 1) * P, :], h_sb[:])

                # --- global amax(h) -> sh, alpha2
                hmax_p = small.tile([P, 1], f32)
                nc.vector.tensor_reduce(hmax_p[:], hmax_cols[:], axis=AX.X, op=A.max)
                stage_h = small.tile([1, 8], f32)
                nc.gpsimd.memset(stage_h[:], 0.0)
                nc.gpsimd.tensor_reduce(stage_h[:, 0:1], hmax_p[:], axis=AX.C, op=A.max)
                nc.sync.dma_start(cc_hmax_in[:, :], stage_h[:])
                nc.gpsimd.collective_compute(
                    "AllReduce", A.max, replica_groups=rg,
                    ins=[cc_hmax_in[:, :]], outs=[cc_hmax_out[:, :]])
                ghmax = small.tile([1, 8], f32)
                nc.sync.dma_start(ghmax[:], cc_hmax_out[:, :])
                scal2 = small.tile([1, 8], f32)
                nc.vector.reciprocal(scal2[:, 0:1], ghmax[:, 0:1])
                nc.vector.tensor_scalar_mul(scal2[:, 0:1], scal2[:, 0:1], 127.0)
                nc.vector.tensor_tensor(scal2[:, 1:2], gsum[:, 1:2], ghmax[:, 0:1], A.mult)
                nc.vector.tensor_scalar_mul(scal2[:, 1:2], scal2[:, 1:2], inv_nw127)
                nc.sync.dma_start(scal2_dram[:, 0:2], scal2[:, 0:2])
                bc2 = cpool.tile([P, 2], f32)
                nc.sync.dma_start(bc2[:], scal2_dram[:, 0:2].to_broadcast((P, 2)))
                sh_b, a2_b = bc2[:, 0:1], bc2[:, 1:2]

            # =========================== phase C: layer 2
            with (
                tc.tile_pool(name="hq", bufs=1) as hqp,
                tc.tile_pool(name="ldh", bufs=3) as ldh,
                tc.tile_pool(name="qh", bufs=3) as qh,
                tc.tile_pool(name="w2g", bufs=3) as w2gp,
                tc.tile_pool(name="ost", bufs=2) as ost,
                tc.tile_pool(name="ps2", bufs=2, space="PSUM") as ps2p,
            ):
                # --- quantize h (from DRAM) -> resident hq (bf16 ints)
                hq = hqp.tile([P, KT2, TOK], bf16)
                for k in range(KT2):
                    t = ldh.tile([P, TOK], f32, tag="lh", name=f"ch{k}")
                    nc.sync.dma_start(t[:], h_dram[k * P:(k + 1) * P, :])
                    q1 = qh.tile([P, TOK], f32, tag="qhh", name=f"cq{k}")
                    nc.scalar.activation(q1[:], t[:], AF.Copy, scale=sh_b)
                    nc.vector.tensor_scalar(
                        hq[:, k, :], q1[:], RND, RND, A.add, A.subtract)

                # --- out.T[m2] = W2q.T @ hq ; scale + bias
                for r2 in range(NCORES):
                    ps2 = {
                        (mh, c): ps2p.tile(
                            [P, NF], f32, tag=f"p2_{mh}_{c}", name=f"p2_{r2}_{mh}_{c}")
                        for mh in range(MPS2) for c in range(NCH)
                    }
                    for g in range(G2):
                        wg = w2gp.tile([P, KG2, DSH], bf16, tag="w2g",
                                       name=f"w2g_{r2}_{g}")
                        base = r2 * F + g * KG2 * P
                        nc.sync.dma_start(
                            wg[:],
                            cc_w2_out[base:base + KG2 * P, :].rearrange(
                                "(k p) d -> p k d", p=P))
                        for kk in range(KG2):
                            k = g * KG2 + kk
                            for mh in range(MPS2):
                                lhsT = wg[:, kk, mh * P:(mh + 1) * P]
                                for c in range(NCH):
                                    nc.tensor.matmul(
                                        ps2[(mh, c)][:], lhsT,
                                        hq[:, k, c * NF:(c + 1) * NF],
                                        start=(k == 0), stop=(k == KT2 - 1))
                    for mh in range(MPS2):
                        m2 = r2 * MPS2 + mh
                        o_sb = ost.tile([P, TOK], f32, tag="osb", name=f"o{m2}")
                        for c in range(NCH):
                            nc.vector.tensor_scalar(
                                o_sb[:, c * NF:(c + 1) * NF], ps2[(mh, c)][:],
                                a2_b, b2_sb[:, m2:m2 + 1], A.mult, A.add)
                        nc.sync.dma_start(outT[m2 * P:(m2 + 1) * P, :], o_sb[:])

    nc.finalize()
    return nc


_NC_CACHE = {}


def _get_nc(T, D, F):
    key = (T, D, F)
    if key not in _NC_CACHE:
        _NC_CACHE[key] = build(T, D, F)
    return _NC_CACHE[key]


LAST_RESULT = None


def kernel(x, W1, b1, W2, b2):
    global LAST_RESULT
    x = np.ascontiguousarray(x, dtype=np.float32)
    W1 = np.ascontiguousarray(W1, dtype=np.float32)
    W2 = np.ascontiguousarray(W2, dtype=np.float32)
    T, D = x.shape
    F = W1.shape[0]
    TOK, FSH, DSH = T // NCORES, F // NCORES, D // NCORES

    nc = _get_nc(T, D, F)

    xT = np.ascontiguousarray(x.T)
    w1T = np.ascontiguousarray(W1.T)
    w2T = np.ascontiguousarray(W2.T)
    b1t = np.ascontiguousarray(b1.astype(np.float32).reshape(F // P, P).T)
    b2t = np.ascontiguousarray(b2.astype(np.float32).reshape(D // P, P).T)

    in_maps = [
        {
            "xT": np.ascontiguousarray(xT[:, r * TOK:(r + 1) * TOK]),
            "w1T": np.ascontiguousarray(w1T[:, r * FSH:(r + 1) * FSH]),
            "w2T": np.ascontiguousarray(w2T[:, r * DSH:(r + 1) * DSH]),
            "b1t": b1t,
            "b2t": b2t,
        }
        for r in range(NCORES)
    ]
    res = run_bass_kernel_spmd(nc, in_maps, core_ids=list(range(NCORES)))
    LAST_RESULT = res
    out_T = np.concatenate([res.results[r]["outT"] for r in range(NCORES)], axis=1)
    return np.ascontiguousarray(out_T.T)


# revision 14
# speedup vs baseline: 22.2323x; 22.2323x over previous
"""BitNet FFN (quantized MLP) on 8 Trainium2 NeuronCores.

Reference computation (per problem nn_BitNetModel_18554258719008):
    h   = quant_act(x) @ tern(W1).T + b1 ; h = gelu(h, exact)
    out = quant_act(h) @ tern(W2).T + b2
  quant_act(v) = round(clip(v*s, -127, 127))/s,  s = 127/max|v| (global)
  tern(w)      = round(clip(w/t, -1, 1))*t,      t = mean|w|    (global)

Key fact: after quantization the matmul operands are exact small integers
(activations in [-127,127], weights in {-1,0,1}) -> bf16 matmul is EXACT.

Sharding (8 cores, SPMD, v2):
  - tokens sharded 8-ways: core r owns tokens [r*1024, (r+1)*1024).
  - every core receives the FULL (host-transposed) W1.T / W2.T and
    ternary-quantizes them locally, streamed slab-by-slab so the quantize
    (DVE/ACT) hides under the matmuls (PE). This avoids the two weight
    AllGathers, which profiling showed cost ~145us each on this fabric.
  - global scales need cross-core reduction: each core also receives a
    distinct 1/8 column-slice of W1.T / W2.T for the |W| sums. One tiny
    AllGather ([1,8] -> [8,8]) carries (amax_x, sum|W1|, sum|W2|); the
    max/add combining happens locally after the gather. A second tiny
    AllGather mid-kernel carries amax(h).
  - everything is fed pre-transposed by the host (pure layout prep) so the
    contraction dim lands on SBUF partitions with no on-device transposes.
    Each core computes h.T for its token shard, then out.T; the host
    reassembles out from the out.T shards.
"""

import numpy as np

import concourse.bacc as bacc_mod
import concourse.mybir as mybir
from concourse.tile import TileContext
from concourse import bass_utils as _bu
from concourse.bass_utils import run_bass_kernel_spmd

# (walrus --enable-ldw-opt rejects bass-emitted InstLdweights; left off)

P = 128
NCORES = 8
RND = 12582912.0  # 1.5 * 2**23: (v + RND) - RND == round-half-even(v) for |v|<2^22

f32 = mybir.dt.float32
bf16 = mybir.dt.bfloat16
A = mybir.AluOpType
AF = mybir.ActivationFunctionType
AX = mybir.AxisListType


def build(T=8192, D=2048, F=8192, gelu=True, timing_mode=False, no_cc=False,
          l1_only=False):
    """Build the SPMD kernel for one core (same program on all 8)."""
    TOK = T // NCORES    # tokens per core
    FSH = F // NCORES    # d_ff columns of W1.T per slab
    DSH = D // NCORES    # d_model columns of W2.T per slab
    KT1 = D // P         # contraction tiles, layer 1
    KT2 = F // P         # contraction tiles, layer 2
    MPS1 = FSH // P      # m-tiles per w1 slab
    MPS2 = DSH // P      # m-tiles per w2 slab
    NF = min(512, TOK)   # matmul moving free width
    NCH = TOK // NF      # chunks of the token axis per matmul row
    KG2 = min(16, KT2)   # k-tiles per w2 quantize/matmul group
    G2 = KT2 // KG2
    inv_nw127 = 1.0 / (D * F) / 127.0

    nc = bacc_mod.Bacc(num_devices=1 if timing_mode else NCORES)

    # ---- I/O (host supplies transposed tensors; see kernel() below)
    xT = nc.dram_tensor("xT", [D, TOK], f32, kind="ExternalInput")
    w1f = nc.dram_tensor("w1f", [D, F], f32, kind="ExternalInput")   # W1.T full
    w2f = nc.dram_tensor("w2f", [F, D], f32, kind="ExternalInput")   # W2.T full
    w1s = nc.dram_tensor("w1s", [D, FSH], f32, kind="ExternalInput")  # my slice
    w2s = nc.dram_tensor("w2s", [F, DSH], f32, kind="ExternalInput")  # my slice
    b1t = nc.dram_tensor("b1t", [P, F // P], f32, kind="ExternalInput")
    b2t = nc.dram_tensor("b2t", [P, D // P], f32, kind="ExternalInput")
    outT = nc.dram_tensor("outT", [D, TOK], f32, kind="ExternalOutput")

    # ---- internal DRAM
    h_dram = nc.dram_tensor("h_dram", [F, TOK], f32)
    cc_stat_in = nc.dram_tensor("cc_stat_in", [1, 8], f32)
    cc_stat_out = nc.dram_tensor("cc_stat_out", [8, 8], f32, addr_space="Shared")
    cc_st2_in = nc.dram_tensor("cc_st2_in", [1, 8], f32)
    cc_st2_out = nc.dram_tensor("cc_st2_out", [8, 8], f32, addr_space="Shared")
    cc_hst_in = nc.dram_tensor("cc_hst_in", [1, 8], f32)
    cc_hst_out = nc.dram_tensor("cc_hst_out", [8, 8], f32, addr_space="Shared")
    scal1_dram = nc.dram_tensor("scal1_dram", [1, 8], f32)
    scal2_dram = nc.dram_tensor("scal2_dram", [1, 8], f32)

    rg = [list(range(NCORES))]

    def cc_ag(in_ap, out_ap):
        if timing_mode or no_cc:
            n = in_ap.shape[0]
            nc.sync.dma_start(out_ap[0:n], in_ap)
        else:
            nc.gpsimd.collective_compute(
                "AllGather", A.bypass, replica_groups=rg,
                ins=[in_ap], outs=[out_ap])

    with TileContext(nc) as tc:
        with (
            tc.tile_pool(name="const", bufs=1) as cpool,
            tc.tile_pool(name="small", bufs=1) as small,
        ):
            # biases (pre-arranged [P, n] by host)
            b1_sb = cpool.tile([P, F // P], f32)
            nc.sync.dma_start(b1_sb[:], b1t[:, :])
            b2_sb = cpool.tile([P, D // P], f32)
            nc.sync.dma_start(b2_sb[:], b2t[:, :])

            # ======= phase A: local reduces + one tiny stats AllGather
            with tc.tile_pool(name="ldA", bufs=4) as ldA:
                xmax_cols = small.tile([P, KT1], f32)
                w1sum_cols = small.tile([P, KT1], f32)
                w2sum_cols = small.tile([P, KT2], f32)
                for i in range(KT1):
                    t = ldA.tile([P, TOK], f32, tag="ldx", name=f"ax{i}")
                    nc.sync.dma_start(t[:], xT[i * P:(i + 1) * P, :])
                    nc.vector.tensor_reduce(
                        xmax_cols[:, i:i + 1], t[:], axis=AX.X, op=A.max,
                        apply_absolute_value=True)
                for i in range(KT1):
                    t = ldA.tile([P, FSH], f32, tag="ldw1", name=f"aw1{i}")
                    nc.sync.dma_start(t[:], w1s[i * P:(i + 1) * P, :])
                    nc.vector.tensor_reduce(
                        w1sum_cols[:, i:i + 1], t[:], axis=AX.X, op=A.add,
                        apply_absolute_value=True)
                for i in range(KT2):
                    t = ldA.tile([P, DSH], f32, tag="ldw2", name=f"aw2{i}")
                    nc.sync.dma_start(t[:], w2s[i * P:(i + 1) * P, :])
                    nc.vector.tensor_reduce(
                        w2sum_cols[:, i:i + 1], t[:], axis=AX.X, op=A.add,
                        apply_absolute_value=True)

                xmax_p = small.tile([P, 1], f32)
                nc.vector.tensor_reduce(xmax_p[:], xmax_cols[:], axis=AX.X, op=A.max)
                w1sum_p = small.tile([P, 1], f32)
                nc.vector.tensor_reduce(w1sum_p[:], w1sum_cols[:], axis=AX.X, op=A.add)
                w2sum_p = small.tile([P, 1], f32)
                nc.vector.tensor_reduce(w2sum_p[:], w2sum_cols[:], axis=AX.X, op=A.add)

                # cross-partition reduce on gpsimd, stage for the AllGathers.
                # stats AG (amax_x, sum|W1|) gates layer 1 -- keep it lean;
                # sum|W2| rides a second AG that is only needed by layer 2.
                stat = small.tile([1, 8], f32)
                nc.gpsimd.memset(stat[:], 0.0)
                nc.gpsimd.tensor_reduce(stat[:, 0:1], xmax_p[:], axis=AX.C, op=A.max)
                nc.gpsimd.tensor_reduce(stat[:, 1:2], w1sum_p[:], axis=AX.C, op=A.add)
                nc.sync.dma_start(cc_stat_in[:, :], stat[:])
                cc_ag(cc_stat_in[:, :], cc_stat_out[:, :])
                stat2 = small.tile([1, 8], f32)
                nc.gpsimd.memset(stat2[:], 0.0)
                nc.gpsimd.tensor_reduce(stat2[:, 0:1], w2sum_p[:], axis=AX.C, op=A.add)
                nc.sync.dma_start(cc_st2_in[:, :], stat2[:])
                cc_ag(cc_st2_in[:, :], cc_st2_out[:, :])

                # gather back as [1, 64] (rank-major) and combine locally:
                # slot s of rank r sits at free offset r*8 + s
                allst = small.tile([1, 64], f32)
                nc.sync.dma_start(
                    allst[:], cc_stat_out[:, :].rearrange("r s -> (r s)")[None, :])
                gmax = small.tile([1, 1], f32)
                nc.vector.tensor_reduce(  # max over ranks of slot 0
                    gmax[:], bass_ap_strided(allst, 0, 1), axis=AX.X, op=A.max)
                gsum = small.tile([1, 2], f32)
                nc.vector.tensor_reduce(  # add over ranks of slot 1 (sum|W1|)
                    gsum[:, 0:1], bass_ap_strided(allst, 1, 1), axis=AX.X, op=A.add)
                allst2 = small.tile([1, 64], f32)
                nc.sync.dma_start(
                    allst2[:], cc_st2_out[:, :].rearrange("r s -> (r s)")[None, :])
                nc.vector.tensor_reduce(  # add over ranks of slot 0 (sum|W2|)
                    gsum[:, 1:2], bass_ap_strided(allst2, 0, 1), axis=AX.X, op=A.add)

                # scalars: sx=127/amax, r1=1/t1, r2=1/t2, alpha1=t1/sx
                scal = small.tile([1, 8], f32)
                nc.vector.reciprocal(scal[:, 0:1], gmax[:, 0:1])
                nc.vector.tensor_scalar_mul(scal[:, 0:1], scal[:, 0:1], 127.0)
                nc.vector.reciprocal(scal[:, 1:2], gsum[:, 0:1])
                nc.vector.tensor_scalar_mul(scal[:, 1:2], scal[:, 1:2], float(D * F))
                nc.vector.reciprocal(scal[:, 2:3], gsum[:, 1:2])
                nc.vector.tensor_scalar_mul(scal[:, 2:3], scal[:, 2:3], float(D * F))
                nc.vector.tensor_tensor(scal[:, 3:4], gsum[:, 0:1], gmax[:, 0:1], A.mult)
                nc.vector.tensor_scalar_mul(scal[:, 3:4], scal[:, 3:4], inv_nw127)
                nc.sync.dma_start(scal1_dram[:, 0:4], scal[:, 0:4])
                bc1 = cpool.tile([P, 4], f32)
                nc.sync.dma_start(bc1[:], scal1_dram[:, 0:4].to_broadcast((P, 4)))
                sx_b, r1_b, r2_b, a1_b = (bc1[:, i:i + 1] for i in range(4))

            # ======= phase B: quantize x + W1 slabs locally, layer 1
            with (
                tc.tile_pool(name="ldB", bufs=3) as ldB,
                tc.tile_pool(name="qt", bufs=3) as qt,
                tc.tile_pool(name="xq", bufs=1) as xqp,
                tc.tile_pool(name="w1s", bufs=2) as w1sp,
                tc.tile_pool(name="hst", bufs=3) as hst,
                tc.tile_pool(name="ps1", bufs=3, space="PSUM") as ps1,
            ):
                # --- quantize x shard -> resident xq (bf16 ints)
                xq = xqp.tile([P, KT1, TOK], bf16)
                for i in range(KT1):
                    t = ldB.tile([P, TOK], f32, tag="lx", name=f"bx{i}")
                    nc.sync.dma_start(t[:], xT[i * P:(i + 1) * P, :])
                    nc.scalar.activation(t[:], t[:], AF.Copy, scale=sx_b)
                    nc.vector.tensor_scalar(
                        xq[:, i, :], t[:], RND, RND, A.add, A.subtract)

                # --- layer 1, slab-major; each slab ternary-quantized locally
                hmax_cols = small.tile([P, F // P], f32)
                act_fn = AF.Gelu if gelu else AF.Identity
                for r in range(NCORES):
                    slab = w1sp.tile([P, KT1, FSH], bf16, tag="w1slab",
                                     name=f"w1q{r}")
                    KC1 = 4
                    for k0 in range(0, KT1, KC1):
                        rt = ldB.tile([P, KC1, FSH], f32, tag="lw1",
                                      name=f"rw1_{r}_{k0}")
                        nc.sync.dma_start(
                            rt[:],
                            w1f[k0 * P:(k0 + KC1) * P,
                                r * FSH:(r + 1) * FSH].rearrange(
                                "(k p) f -> p k f", p=P))
                        # tern: round(clamp(w*r1)) == clamp(round(w*r1))
                        nc.scalar.activation(rt[:], rt[:], AF.Copy, scale=r1_b)
                        nc.vector.tensor_scalar(rt[:], rt[:], RND, RND,
                                                A.add, A.subtract)
                        nc.vector.tensor_scalar(slab[:, k0:k0 + KC1, :], rt[:],
                                                1.0, -1.0, A.min, A.max)
                    for j in range(MPS1):
                        m = r * MPS1 + j
                        ps = ps1.tile([P, TOK], f32, tag="ps", name=f"ps_{m}")
                        for k in range(KT1):
                            lhsT = slab[:, k, j * P:(j + 1) * P]
                            for c in range(NCH):
                                nc.tensor.matmul(
                                    ps[:, c * NF:(c + 1) * NF], lhsT,
                                    xq[:, k, c * NF:(c + 1) * NF],
                                    start=(k == 0), stop=(k == KT1 - 1))
                        h_sb = hst.tile([P, TOK], f32, tag="hsb", name=f"h{m}")
                        nc.scalar.activation(
                            h_sb[:], ps[:], act_fn,
                            bias=b1_sb[:, m:m + 1], scale=a1_b)
                        nc.vector.tensor_reduce(
                            hmax_cols[:, m:m + 1], h_sb[:], axis=AX.X, op=A.max,
                            apply_absolute_value=True)
                        nc.sync.dma_start(h_dram[m * P:(m + 1) * P, :], h_sb[:])

                # --- global amax(h) -> sh, alpha2 (one tiny AllGather)
                hmax_p = small.tile([P, 1], f32)
                nc.vector.tensor_reduce(hmax_p[:], hmax_cols[:], axis=AX.X, op=A.max)
                stage_h = small.tile([1, 8], f32)
                nc.gpsimd.memset(stage_h[:], 0.0)
                nc.gpsimd.tensor_reduce(
                    stage_h[:, 0:1], hmax_p[:], axis=AX.C, op=A.max)
                nc.sync.dma_start(cc_hst_in[:, :], stage_h[:])
                cc_ag(cc_hst_in[:, :], cc_hst_out[:, :])
                allh = small.tile([1, 64], f32)
                nc.sync.dma_start(
                    allh[:], cc_hst_out[:, :].rearrange("r s -> (r s)")[None, :])
                ghmax = small.tile([1, 1], f32)
                nc.vector.tensor_reduce(
                    ghmax[:], bass_ap_strided(allh, 0, 1), axis=AX.X, op=A.max)
                scal2 = small.tile([1, 8], f32)
                nc.vector.reciprocal(scal2[:, 0:1], ghmax[:, 0:1])
                nc.vector.tensor_scalar_mul(scal2[:, 0:1], scal2[:, 0:1], 127.0)
                nc.vector.tensor_tensor(
                    scal2[:, 1:2], gsum[:, 1:2], ghmax[:, 0:1], A.mult)
                nc.vector.tensor_scalar_mul(scal2[:, 1:2], scal2[:, 1:2], inv_nw127)
                nc.sync.dma_start(scal2_dram[:, 0:2], scal2[:, 0:2])
                bc2 = cpool.tile([P, 2], f32)
                nc.sync.dma_start(bc2[:], scal2_dram[:, 0:2].to_broadcast((P, 2)))
                sh_b, a2_b = bc2[:, 0:1], bc2[:, 1:2]

            # ======= phase C: layer 2 (W2 quantized locally per slab)
            if not l1_only:
                with (
                    tc.tile_pool(name="hq", bufs=1) as hqp,
                    tc.tile_pool(name="ldh", bufs=4) as ldh,
                    tc.tile_pool(name="qh", bufs=2) as qh,
                    tc.tile_pool(name="ldw2", bufs=2) as ldw2,
                    tc.tile_pool(name="q2t", bufs=3) as q2t,
                    tc.tile_pool(name="w2g", bufs=2) as w2gp,
                    tc.tile_pool(name="ost", bufs=2) as ost,
                    tc.tile_pool(name="ps2", bufs=2, space="PSUM") as ps2p,
                ):
                    # --- quantize h (from DRAM) -> resident hq (bf16 ints)
                    hq = hqp.tile([P, KT2, TOK], bf16)
                    for k in range(KT2):
                        t = ldh.tile([P, TOK], f32, tag="lh", name=f"ch{k}")
                        nc.sync.dma_start(t[:], h_dram[k * P:(k + 1) * P, :])
                        nc.scalar.activation(t[:], t[:], AF.Copy, scale=sh_b)
                        nc.vector.tensor_scalar(
                            hq[:, k, :], t[:], RND, RND, A.add, A.subtract)

                    # --- out.T[m2] = W2q.T @ hq ; scale + bias
                    for r2 in range(NCORES):
                        ps2 = {
                            mh: ps2p.tile(
                                [P, TOK], f32, tag=f"p2_{mh}",
                                name=f"p2_{r2}_{mh}")
                            for mh in range(MPS2)
                        }
                        for g in range(G2):
                            wg = w2gp.tile([P, KG2, DSH], bf16, tag="w2g",
                                           name=f"w2g_{r2}_{g}")
                            KC2 = 8
                            for kk0 in range(0, KG2, KC2):
                                k0 = g * KG2 + kk0
                                rt = ldw2.tile([P, KC2, DSH], f32, tag="lw2",
                                               name=f"rw2_{r2}_{g}_{kk0}")
                                nc.sync.dma_start(
                                    rt[:],
                                    w2f[k0 * P:(k0 + KC2) * P,
                                        r2 * DSH:(r2 + 1) * DSH].rearrange(
                                        "(k p) d -> p k d", p=P))
                                nc.scalar.activation(rt[:], rt[:], AF.Copy,
                                                     scale=r2_b)
                                nc.vector.tensor_scalar(rt[:], rt[:], RND, RND,
                                                        A.add, A.subtract)
                                nc.vector.tensor_scalar(
                                    wg[:, kk0:kk0 + KC2, :], rt[:],
                                    1.0, -1.0, A.min, A.max)
                            for kk in range(KG2):
                                k = g * KG2 + kk
                                for mh in range(MPS2):
                                    lhsT = wg[:, kk, mh * P:(mh + 1) * P]
                                    for c2 in range(NCH):
                                        nc.tensor.matmul(
                                            ps2[mh][:, c2 * NF:(c2 + 1) * NF],
                                            lhsT,
                                            hq[:, k, c2 * NF:(c2 + 1) * NF],
                                            start=(k == 0), stop=(k == KT2 - 1))
                        for mh in range(MPS2):
                            m2 = r2 * MPS2 + mh
                            o_sb = ost.tile([P, TOK], f32, tag="osb", name=f"o{m2}")
                            nc.vector.tensor_scalar(
                                o_sb[:], ps2[mh][:],
                                a2_b, b2_sb[:, m2:m2 + 1], A.mult, A.add)
                            nc.sync.dma_start(outT[m2 * P:(m2 + 1) * P, :], o_sb[:])

    nc.finalize()
    return nc


def bass_ap_strided(tile, slot0, nslots):
    """AP over a [1, 64] rank-major stats tile selecting slots
    [slot0, slot0+nslots) per rank, ranks innermost: [1, nslots, 8ranks].
    reduce(axis=X) then collapses the rank dim."""
    return tile[:, :].rearrange(
        "p (r s) -> p s r", r=8, s=8)[:, slot0:slot0 + nslots, :]


_NC_CACHE = {}


def _get_nc(T, D, F, **kw):
    key = (T, D, F, tuple(sorted(kw.items())))
    if key not in _NC_CACHE:
        _NC_CACHE[key] = build(T, D, F, **kw)
    return _NC_CACHE[key]


LAST_RESULT = None


def kernel(x, W1, b1, W2, b2):
    global LAST_RESULT
    x = np.ascontiguousarray(x, dtype=np.float32)
    W1 = np.ascontiguousarray(W1, dtype=np.float32)
    W2 = np.ascontiguousarray(W2, dtype=np.float32)
    T, D = x.shape
    F = W1.shape[0]
    TOK, FSH, DSH = T // NCORES, F // NCORES, D // NCORES

    nc = _get_nc(T, D, F)

    xT = np.ascontiguousarray(x.T)
    w1f = np.ascontiguousarray(W1.T)
    w2f = np.ascontiguousarray(W2.T)
    b1t = np.ascontiguousarray(b1.astype(np.float32).reshape(F // P, P).T)
    b2t = np.ascontiguousarray(b2.astype(np.float32).reshape(D // P, P).T)

    in_maps = [
        {
            "xT": np.ascontiguousarray(xT[:, r * TOK:(r + 1) * TOK]),
            "w1f": w1f,
            "w2f": w2f,
            "w1s": np.ascontiguousarray(w1f[:, r * FSH:(r + 1) * FSH]),
            "w2s": np.ascontiguousarray(w2f[:, r * DSH:(r + 1) * DSH]),
            "b1t": b1t,
            "b2t": b2t,
        }
        for r in range(NCORES)
    ]
    res = run_bass_kernel_spmd(nc, in_maps, core_ids=list(range(NCORES)))
    LAST_RESULT = res
    out_T = np.concatenate([res.results[r]["outT"] for r in range(NCORES)], axis=1)
    return np.ascontiguousarray(out_T.T)


# revision 15
# speedup vs baseline: 22.5303x; 1.0134x over previous
"""BitNet FFN (quantized MLP) on 8 Trainium2 NeuronCores.

Reference computation (per problem nn_BitNetModel_18554258719008):
    h   = quant_act(x) @ tern(W1).T + b1 ; h = gelu(h, exact)
    out = quant_act(h) @ tern(W2).T + b2
  quant_act(v) = round(clip(v*s, -127, 127))/s,  s = 127/max|v| (global)
  tern(w)      = round(clip(w/t, -1, 1))*t,      t = mean|w|    (global)

Key fact: after quantization the matmul operands are exact small integers
(activations in [-127,127], weights in {-1,0,1}) -> bf16 matmul is EXACT.

Sharding (8 cores, SPMD, v2):
  - tokens sharded 8-ways: core r owns tokens [r*1024, (r+1)*1024).
  - every core receives the FULL (host-transposed) W1.T / W2.T and
    ternary-quantizes them locally, streamed slab-by-slab so the quantize
    (DVE/ACT) hides under the matmuls (PE). This avoids the two weight
    AllGathers, which profiling showed cost ~145us each on this fabric.
  - global scales need cross-core reduction: each core also receives a
    distinct 1/8 column-slice of W1.T / W2.T for the |W| sums. One tiny
    AllGather ([1,8] -> [8,8]) carries (amax_x, sum|W1|, sum|W2|); the
    max/add combining happens locally after the gather. A second tiny
    AllGather mid-kernel carries amax(h).
  - everything is fed pre-transposed by the host (pure layout prep) so the
    contraction dim lands on SBUF partitions with no on-device transposes.
    Each core computes h.T for its token shard, then out.T; the host
    reassembles out from the out.T shards.
"""

import numpy as np

import concourse.bacc as bacc_mod
import concourse.mybir as mybir
from concourse.tile import TileContext
from concourse import bass_utils as _bu
from concourse.bass_utils import run_bass_kernel_spmd

# (walrus --enable-ldw-opt rejects bass-emitted InstLdweights; left off)

P = 128
NCORES = 8
RND = 12582912.0  # 1.5 * 2**23: (v + RND) - RND == round-half-even(v) for |v|<2^22

f32 = mybir.dt.float32
bf16 = mybir.dt.bfloat16
A = mybir.AluOpType
AF = mybir.ActivationFunctionType
AX = mybir.AxisListType


def build(T=8192, D=2048, F=8192, gelu=True, timing_mode=False, no_cc=False,
          l1_only=False):
    """Build the SPMD kernel for one core (same program on all 8)."""
    TOK = T // NCORES    # tokens per core
    FSH = F // NCORES    # d_ff columns of W1.T per slab
    DSH = D // NCORES    # d_model columns of W2.T per slab
    KT1 = D // P         # contraction tiles, layer 1
    KT2 = F // P         # contraction tiles, layer 2
    MPS1 = FSH // P      # m-tiles per w1 slab
    MPS2 = DSH // P      # m-tiles per w2 slab
    NF = min(512, TOK)   # matmul moving free width
    NCH = TOK // NF      # chunks of the token axis per matmul row
    KG2 = min(16, KT2)   # k-tiles per w2 quantize/matmul group
    G2 = KT2 // KG2
    inv_nw127 = 1.0 / (D * F) / 127.0

    nc = bacc_mod.Bacc(num_devices=1 if timing_mode else NCORES)

    # ---- I/O (host supplies transposed tensors; see kernel() below)
    xT = nc.dram_tensor("xT", [D, TOK], f32, kind="ExternalInput")
    w1f = nc.dram_tensor("w1f", [D, F], f32, kind="ExternalInput")   # W1.T full
    w2f = nc.dram_tensor("w2f", [F, D], f32, kind="ExternalInput")   # W2.T full
    w1s = nc.dram_tensor("w1s", [D, FSH], f32, kind="ExternalInput")  # my slice
    w2s = nc.dram_tensor("w2s", [F, DSH], f32, kind="ExternalInput")  # my slice
    b1t = nc.dram_tensor("b1t", [P, F // P], f32, kind="ExternalInput")
    b2t = nc.dram_tensor("b2t", [P, D // P], f32, kind="ExternalInput")
    outT = nc.dram_tensor("outT", [D, TOK], f32, kind="ExternalOutput")

    # ---- internal DRAM
    h_dram = nc.dram_tensor("h_dram", [F, TOK], f32)
    cc_stat_in = nc.dram_tensor("cc_stat_in", [1, 8], f32)
    cc_stat_out = nc.dram_tensor("cc_stat_out", [8, 8], f32, addr_space="Shared")
    cc_st2_in = nc.dram_tensor("cc_st2_in", [1, 8], f32)
    cc_st2_out = nc.dram_tensor("cc_st2_out", [8, 8], f32, addr_space="Shared")
    cc_hst_in = nc.dram_tensor("cc_hst_in", [1, 8], f32)
    cc_hst_out = nc.dram_tensor("cc_hst_out", [8, 8], f32, addr_space="Shared")
    scal1_dram = nc.dram_tensor("scal1_dram", [1, 8], f32)
    scal2_dram = nc.dram_tensor("scal2_dram", [1, 8], f32)

    rg = [list(range(NCORES))]

    def cc_ag(in_ap, out_ap):
        if timing_mode or no_cc:
            n = in_ap.shape[0]
            nc.sync.dma_start(out_ap[0:n], in_ap)
        else:
            nc.gpsimd.collective_compute(
                "AllGather", A.bypass, replica_groups=rg,
                ins=[in_ap], outs=[out_ap])

    with TileContext(nc) as tc:
        with (
            tc.tile_pool(name="const", bufs=1) as cpool,
            tc.tile_pool(name="small", bufs=1) as small,
        ):
            # biases (pre-arranged [P, n] by host)
            b1_sb = cpool.tile([P, F // P], f32)
            nc.sync.dma_start(b1_sb[:], b1t[:, :])
            b2_sb = cpool.tile([P, D // P], f32)
            nc.sync.dma_start(b2_sb[:], b2t[:, :])

            # ======= phase A: local reduces + one tiny stats AllGather
            with tc.tile_pool(name="ldA", bufs=4) as ldA:
                xmax_cols = small.tile([P, KT1], f32)
                w1sum_cols = small.tile([P, KT1], f32)
                w2sum_cols = small.tile([P, KT2], f32)
                for i in range(KT1):
                    t = ldA.tile([P, TOK], f32, tag="ldx", name=f"ax{i}")
                    nc.sync.dma_start(t[:], xT[i * P:(i + 1) * P, :])
                    nc.vector.tensor_reduce(
                        xmax_cols[:, i:i + 1], t[:], axis=AX.X, op=A.max,
                        apply_absolute_value=True)
                for i in range(KT1):
                    t = ldA.tile([P, FSH], f32, tag="ldw1", name=f"aw1{i}")
                    nc.sync.dma_start(t[:], w1s[i * P:(i + 1) * P, :])
                    nc.vector.tensor_reduce(
                        w1sum_cols[:, i:i + 1], t[:], axis=AX.X, op=A.add,
                        apply_absolute_value=True)
                for i in range(KT2):
                    t = ldA.tile([P, DSH], f32, tag="ldw2", name=f"aw2{i}")
                    nc.sync.dma_start(t[:], w2s[i * P:(i + 1) * P, :])
                    nc.vector.tensor_reduce(
                        w2sum_cols[:, i:i + 1], t[:], axis=AX.X, op=A.add,
                        apply_absolute_value=True)

                xmax_p = small.tile([P, 1], f32)
                nc.vector.tensor_reduce(xmax_p[:], xmax_cols[:], axis=AX.X, op=A.max)
                w1sum_p = small.tile([P, 1], f32)
                nc.vector.tensor_reduce(w1sum_p[:], w1sum_cols[:], axis=AX.X, op=A.add)
                w2sum_p = small.tile([P, 1], f32)
                nc.vector.tensor_reduce(w2sum_p[:], w2sum_cols[:], axis=AX.X, op=A.add)

                # cross-partition reduce on gpsimd, stage for the AllGathers.
                # stats AG (amax_x, sum|W1|) gates layer 1 -- keep it lean;
                # sum|W2| rides a second AG that is only needed by layer 2.
                stat = small.tile([1, 8], f32)
                nc.gpsimd.memset(stat[:], 0.0)
                nc.gpsimd.tensor_reduce(stat[:, 0:1], xmax_p[:], axis=AX.C, op=A.max)
                nc.gpsimd.tensor_reduce(stat[:, 1:2], w1sum_p[:], axis=AX.C, op=A.add)
                nc.sync.dma_start(cc_stat_in[:, :], stat[:])
                cc_ag(cc_stat_in[:, :], cc_stat_out[:, :])
                stat2 = small.tile([1, 8], f32)
                nc.gpsimd.memset(stat2[:], 0.0)
                nc.gpsimd.tensor_reduce(stat2[:, 0:1], w2sum_p[:], axis=AX.C, op=A.add)
                nc.sync.dma_start(cc_st2_in[:, :], stat2[:])
                cc_ag(cc_st2_in[:, :], cc_st2_out[:, :])

                # gather back as [1, 64] (rank-major) and combine locally:
                # slot s of rank r sits at free offset r*8 + s
                allst = small.tile([1, 64], f32)
                nc.sync.dma_start(
                    allst[:], cc_stat_out[:, :].rearrange("r s -> (r s)")[None, :])
                gmax = small.tile([1, 1], f32)
                nc.vector.tensor_reduce(  # max over ranks of slot 0
                    gmax[:], bass_ap_strided(allst, 0, 1), axis=AX.X, op=A.max)
                gsum = small.tile([1, 2], f32)
                nc.vector.tensor_reduce(  # add over ranks of slot 1 (sum|W1|)
                    gsum[:, 0:1], bass_ap_strided(allst, 1, 1), axis=AX.X, op=A.add)
                allst2 = small.tile([1, 64], f32)
                nc.sync.dma_start(
                    allst2[:], cc_st2_out[:, :].rearrange("r s -> (r s)")[None, :])
                nc.vector.tensor_reduce(  # add over ranks of slot 0 (sum|W2|)
                    gsum[:, 1:2], bass_ap_strided(allst2, 0, 1), axis=AX.X, op=A.add)

                # scalars: sx=127/amax, r1=1/t1, r2=1/t2, alpha1=t1/sx
                scal = small.tile([1, 8], f32)
                nc.vector.reciprocal(scal[:, 0:1], gmax[:, 0:1])
                nc.vector.tensor_scalar_mul(scal[:, 0:1], scal[:, 0:1], 127.0)
                nc.vector.reciprocal(scal[:, 1:2], gsum[:, 0:1])
                nc.vector.tensor_scalar_mul(scal[:, 1:2], scal[:, 1:2], float(D * F))
                nc.vector.reciprocal(scal[:, 2:3], gsum[:, 1:2])
                nc.vector.tensor_scalar_mul(scal[:, 2:3], scal[:, 2:3], float(D * F))
                nc.vector.tensor_tensor(scal[:, 3:4], gsum[:, 0:1], gmax[:, 0:1], A.mult)
                nc.vector.tensor_scalar_mul(scal[:, 3:4], scal[:, 3:4], inv_nw127)
                nc.sync.dma_start(scal1_dram[:, 0:4], scal[:, 0:4])
                bc1 = cpool.tile([P, 4], f32)
                nc.sync.dma_start(bc1[:], scal1_dram[:, 0:4].to_broadcast((P, 4)))
                sx_b, r1_b, r2_b, a1_b = (bc1[:, i:i + 1] for i in range(4))

            # ======= phase B: quantize x + W1 slabs locally, layer 1
            with (
                tc.tile_pool(name="ldB", bufs=3) as ldB,
                tc.tile_pool(name="qt", bufs=3) as qt,
                tc.tile_pool(name="xq", bufs=1) as xqp,
                tc.tile_pool(name="w1s", bufs=2) as w1sp,
                tc.tile_pool(name="hst", bufs=3) as hst,
                tc.tile_pool(name="ps1", bufs=3, space="PSUM") as ps1,
            ):
                # --- quantize x shard + W1 slab 0, interleaved k-aligned so
                # the first matmul group (needs xq[k] AND slab0[k] in k order)
                # is gated by neither stream finishing entirely.
                xq = xqp.tile([P, KT1, TOK], bf16)
                KC1 = 4

                def w1_quant_chunk(slab, r, k0):
                    rt = ldB.tile([P, KC1, FSH], f32, tag="lw1",
                                  name=f"rw1_{r}_{k0}")
                    nc.sync.dma_start(
                        rt[:],
                        w1f[k0 * P:(k0 + KC1) * P,
                            r * FSH:(r + 1) * FSH].rearrange(
                            "(k p) f -> p k f", p=P))
                    # tern: round(clamp(w*r1)) == clamp(round(w*r1))
                    nc.scalar.activation(rt[:], rt[:], AF.Copy, scale=r1_b)
                    nc.vector.tensor_scalar(rt[:], rt[:], RND, RND,
                                            A.add, A.subtract)
                    nc.vector.tensor_scalar(slab[:, k0:k0 + KC1, :], rt[:],
                                            1.0, -1.0, A.min, A.max)

                slab0 = w1sp.tile([P, KT1, FSH], bf16, tag="w1slab",
                                  name="w1q0")
                for k0 in range(0, KT1, KC1):
                    w1_quant_chunk(slab0, 0, k0)
                    for i in range(k0, k0 + KC1):
                        t = ldB.tile([P, TOK], f32, tag="lx", name=f"bx{i}")
                        nc.sync.dma_start(t[:], xT[i * P:(i + 1) * P, :])
                        nc.scalar.activation(t[:], t[:], AF.Copy, scale=sx_b)
                        nc.vector.tensor_scalar(
                            xq[:, i, :], t[:], RND, RND, A.add, A.subtract)

                # --- layer 1, slab-major; each slab ternary-quantized locally
                hmax_cols = small.tile([P, F // P], f32)
                act_fn = AF.Gelu if gelu else AF.Identity
                for r in range(NCORES):
                    if r == 0:
                        slab = slab0
                    else:
                        slab = w1sp.tile([P, KT1, FSH], bf16, tag="w1slab",
                                         name=f"w1q{r}")
                        for k0 in range(0, KT1, KC1):
                            w1_quant_chunk(slab, r, k0)
                    for j in range(MPS1):
                        m = r * MPS1 + j
                        ps = ps1.tile([P, TOK], f32, tag="ps", name=f"ps_{m}")
                        for k in range(KT1):
                            lhsT = slab[:, k, j * P:(j + 1) * P]
                            for c in range(NCH):
                                nc.tensor.matmul(
                                    ps[:, c * NF:(c + 1) * NF], lhsT,
                                    xq[:, k, c * NF:(c + 1) * NF],
                                    start=(k == 0), stop=(k == KT1 - 1))
                        h_sb = hst.tile([P, TOK], f32, tag="hsb", name=f"h{m}")
                        nc.scalar.activation(
                            h_sb[:], ps[:], act_fn,
                            bias=b1_sb[:, m:m + 1], scale=a1_b)
                        nc.vector.tensor_reduce(
                            hmax_cols[:, m:m + 1], h_sb[:], axis=AX.X, op=A.max,
                            apply_absolute_value=True)
                        nc.sync.dma_start(h_dram[m * P:(m + 1) * P, :], h_sb[:])

                # --- global amax(h) -> sh, alpha2 (one tiny AllGather)
                hmax_p = small.tile([P, 1], f32)
                nc.vector.tensor_reduce(hmax_p[:], hmax_cols[:], axis=AX.X, op=A.max)
                stage_h = small.tile([1, 8], f32)
                nc.gpsimd.memset(stage_h[:], 0.0)
                nc.gpsimd.tensor_reduce(
                    stage_h[:, 0:1], hmax_p[:], axis=AX.C, op=A.max)
                nc.sync.dma_start(cc_hst_in[:, :], stage_h[:])
                cc_ag(cc_hst_in[:, :], cc_hst_out[:, :])
                allh = small.tile([1, 64], f32)
                nc.sync.dma_start(
                    allh[:], cc_hst_out[:, :].rearrange("r s -> (r s)")[None, :])
                ghmax = small.tile([1, 1], f32)
                nc.vector.tensor_reduce(
                    ghmax[:], bass_ap_strided(allh, 0, 1), axis=AX.X, op=A.max)
                scal2 = small.tile([1, 8], f32)
                nc.vector.reciprocal(scal2[:, 0:1], ghmax[:, 0:1])
                nc.vector.tensor_scalar_mul(scal2[:, 0:1], scal2[:, 0:1], 127.0)
                nc.vector.tensor_tensor(
                    scal2[:, 1:2], gsum[:, 1:2], ghmax[:, 0:1], A.mult)
                nc.vector.tensor_scalar_mul(scal2[:, 1:2], scal2[:, 1:2], inv_nw127)
                nc.sync.dma_start(scal2_dram[:, 0:2], scal2[:, 0:2])
                bc2 = cpool.tile([P, 2], f32)
                nc.sync.dma_start(bc2[:], scal2_dram[:, 0:2].to_broadcast((P, 2)))
                sh_b, a2_b = bc2[:, 0:1], bc2[:, 1:2]

            # ======= phase C: layer 2 (W2 quantized locally per slab)
            if not l1_only:
                with (
                    tc.tile_pool(name="hq", bufs=1) as hqp,
                    tc.tile_pool(name="ldh", bufs=6) as ldh,
                    tc.tile_pool(name="qh", bufs=2) as qh,
                    tc.tile_pool(name="ldw2", bufs=2) as ldw2,
                    tc.tile_pool(name="q2t", bufs=3) as q2t,
                    tc.tile_pool(name="w2g", bufs=2) as w2gp,
                    tc.tile_pool(name="ost", bufs=2) as ost,
                    tc.tile_pool(name="ps2", bufs=2, space="PSUM") as ps2p,
                ):
                    # --- quantize h (from DRAM) -> resident hq (bf16 ints)
                    hq = hqp.tile([P, KT2, TOK], bf16)
                    for k in range(KT2):
                        t = ldh.tile([P, TOK], f32, tag="lh", name=f"ch{k}")
                        nc.sync.dma_start(t[:], h_dram[k * P:(k + 1) * P, :])
                        nc.scalar.activation(t[:], t[:], AF.Copy, scale=sh_b)
                        nc.vector.tensor_scalar(
                            hq[:, k, :], t[:], RND, RND, A.add, A.subtract)

                    # --- out.T[m2] = W2q.T @ hq ; scale + bias
                    for r2 in range(NCORES):
                        ps2 = {
                            mh: ps2p.tile(
                                [P, TOK], f32, tag=f"p2_{mh}",
                                name=f"p2_{r2}_{mh}")
                            for mh in range(MPS2)
                        }
                        for g in range(G2):
                            wg = w2gp.tile([P, KG2, DSH], bf16, tag="w2g",
                                           name=f"w2g_{r2}_{g}")
                            KC2 = 8
                            for kk0 in range(0, KG2, KC2):
                                k0 = g * KG2 + kk0
                                rt = ldw2.tile([P, KC2, DSH], f32, tag="lw2",
                                               name=f"rw2_{r2}_{g}_{kk0}")
                                nc.sync.dma_start(
                                    rt[:],
                                    w2f[k0 * P:(k0 + KC2) * P,
                                        r2 * DSH:(r2 + 1) * DSH].rearrange(
                                        "(k p) d -> p k d", p=P))
                                nc.scalar.activation(rt[:], rt[:], AF.Copy,
                                                     scale=r2_b)
                                nc.vector.tensor_scalar(rt[:], rt[:], RND, RND,
                                                        A.add, A.subtract)
                                nc.vector.tensor_scalar(
                                    wg[:, kk0:kk0 + KC2, :], rt[:],
                                    1.0, -1.0, A.min, A.max)
                            for kk in range(KG2):
                                k = g * KG2 + kk
                                for mh in range(MPS2):
                                    lhsT = wg[:, kk, mh * P:(mh + 1) * P]
                                    for c2 in range(NCH):
                                        nc.tensor.matmul(
                                            ps2[mh][:, c2 * NF:(c2 + 1) * NF],
                                            lhsT,
                                            hq[:, k, c2 * NF:(c2 + 1) * NF],
                                            start=(k == 0), stop=(k == KT2 - 1))
                        for mh in range(MPS2):
                            m2 = r2 * MPS2 + mh
                            o_sb = ost.tile([P, TOK], f32, tag="osb", name=f"o{m2}")
                            nc.vector.tensor_scalar(
                                o_sb[:], ps2[mh][:],
                                a2_b, b2_sb[:, m2:m2 + 1], A.mult, A.add)
                            nc.sync.dma_start(outT[m2 * P:(m2 + 1) * P, :], o_sb[:])

    nc.finalize()
    return nc


def bass_ap_strided(tile, slot0, nslots):
    """AP over a [1, 64] rank-major stats tile selecting slots
    [slot0, slot0+nslots) per rank, ranks innermost: [1, nslots, 8ranks].
    reduce(axis=X) then collapses the rank dim."""
    return tile[:, :].rearrange(
        "p (r s) -> p s r", r=8, s=8)[:, slot0:slot0 + nslots, :]


_NC_CACHE = {}


def _get_nc(T, D, F, **kw):
    key = (T, D, F, tuple(sorted(kw.items())))
    if key not in _NC_CACHE:
        _NC_CACHE[key] = build(T, D, F, **kw)
    return _NC_CACHE[key]


LAST_RESULT = None


def kernel(x, W1, b1, W2, b2):
    global LAST_RESULT
    x = np.ascontiguousarray(x, dtype=np.float32)
    W1 = np.ascontiguousarray(W1, dtype=np.float32)
    W2 = np.ascontiguousarray(W2, dtype=np.float32)
    T, D = x.shape
    F = W1.shape[0]
    TOK, FSH, DSH = T // NCORES, F // NCORES, D // NCORES

    nc = _get_nc(T, D, F)

    xT = np.ascontiguousarray(x.T)
    w1f = np.ascontiguousarray(W1.T)
    w2f = np.ascontiguousarray(W2.T)
    b1t = np.ascontiguousarray(b1.astype(np.float32).reshape(F // P, P).T)
    b2t = np.ascontiguousarray(b2.astype(np.float32).reshape(D // P, P).T)

    in_maps = [
        {
            "xT": np.ascontiguousarray(xT[:, r * TOK:(r + 1) * TOK]),
            "w1f": w1f,
            "w2f": w2f,
            "w1s": np.ascontiguousarray(w1f[:, r * FSH:(r + 1) * FSH]),
            "w2s": np.ascontiguousarray(w2f[:, r * DSH:(r + 1) * DSH]),
            "b1t": b1t,
            "b2t": b2t,
        }
        for r in range(NCORES)
    ]
    res = run_bass_kernel_spmd(nc, in_maps, core_ids=list(range(NCORES)))
    LAST_RESULT = res
    out_T = np.concatenate([res.results[r]["outT"] for r in range(NCORES)], axis=1)
    return np.ascontiguousarray(out_T.T)
